# revision 8
# baseline (speedup 1.0000x reference)
"""IsoMaxPlus first-part kernel for TRN2 (8 NeuronCores, data-parallel on B).

out[b, c] = -|s| * sqrt(max(2 - 2 * <f_b/||f_b||, p_c/||p_c||>, 1e-12))

v2 strategy per core (B-shard of 8192 rows), built around three fixes to the
v1 trace (730us): DMA-instruction serialization on the Sync queue (288 small
DMA transposes = 349us busy), cold PE (HAM rethrottle to 1.2GHz), and tiny
HBM transfers.

  prolog: prototypes (host-padded to [1024, 512]) loaded as 2x [128,4,512]
          fp32 supergroups, row-normalized (ACT square+accum, DVE recip),
          cast bf16, and transposed with ONE xbar DMA-transpose per
          supergroup into pnT [128, sg:2, jc:4, k:4, c:128] (the HW xbar
          writes [128,128] chunk-transposes contiguously from the base).
  main:   16 groups of 512 feature rows (4 blocks of 128):
          - one 1MB SWDGE load  [128,4,512] fp32   (gpsimd queue)
          - DVE: bf16 cast + scalar_tensor_tensor square w/ accum row-norms
            (small [128,4] ops batched per group)
          - ONE xbar DMA transpose [128,2048]bf16 -> fT 16 chunks of
            [128,128] (sync queue; chunk 4j+k = block j, d-chunk k)
          - PE: per block 4k x 2 halves of [K=128,N=512] bf16 matmuls into a
            [128,1024] psum tile (c padded; pad protos are zero), blocks
            back-to-back so HAM stays at 2.4GHz
          - ACT: Sqrt(scale*x+bias) epilogue fusing f-norm + distance_scale
          - gpsimd: negate
          - one 2MB store [128,4,1000] fp32 (sync queue)
Engine budget per group (~9us pipelined): HBM ~8.8us, PE ~7us, ACT ~5.8us,
DVE ~3.5us, gpsimd ~4.6us, sync-queue ~4us -> HBM-bound near ~150us/core.
"""

import numpy as np
from contextlib import ExitStack

import concourse.bass as bass
import concourse.tile as tile
from concourse import bacc, mybir
from concourse.bass import ts
from concourse.bass_utils import run_bass_kernel_spmd

N_CORES = 8
B, D, C = 65536, 512, 1000
CP = 1024                  # prototypes padded (zeros) for transpose/psum align
BS = B // N_CORES          # 8192 rows per core
G = 4                      # feature blocks (128 rows) per group
NG = BS // (G * 128)       # 16 groups
KC = D // 128              # 4 contraction chunks
F32 = mybir.dt.float32
BF16 = mybir.dt.bfloat16


def _emit(nc):
    f_dram = nc.dram_tensor("features", [BS, D], F32, kind="ExternalInput").ap()
    p_dram = nc.dram_tensor("prototypes", [CP, D], F32, kind="ExternalInput").ap()
    s_dram = nc.dram_tensor("distance_scale", [1], F32, kind="ExternalInput").ap()
    o_dram = nc.dram_tensor("out", [BS, C], F32, kind="ExternalOutput").ap()

    mult = mybir.AluOpType.mult

    with tile.TileContext(nc) as tc, ExitStack() as ctx:
        singles = ctx.enter_context(tc.tile_pool(name="singles", bufs=1))
        pstage = ctx.enter_context(tc.tile_pool(name="pstage", bufs=2))
        psq = ctx.enter_context(tc.tile_pool(name="psq", bufs=2))
        pnbp = ctx.enter_context(tc.tile_pool(name="pnbp", bufs=2))
        fpool = ctx.enter_context(tc.tile_pool(name="fpool", bufs=3))
        fbpool = ctx.enter_context(tc.tile_pool(name="fbpool", bufs=2))
        sqpool = ctx.enter_context(tc.tile_pool(name="sqpool", bufs=2))
        ftpool = ctx.enter_context(tc.tile_pool(name="ftpool", bufs=3))
        opool = ctx.enter_context(tc.tile_pool(name="opool", bufs=2))
        small = ctx.enter_context(tc.tile_pool(name="small", bufs=2))
        mpsum = ctx.enter_context(tc.tile_pool(name="mpsum", bufs=3, space="PSUM"))

        # distance_scale -> per-partition constants 2*s^2 and -2*s^2
        s_b = singles.tile([128, 1], F32)
        nc.gpsimd.dma_start(out=s_b[:], in_=s_dram.to_broadcast([128, 1]))
        s2 = singles.tile([128, 1], F32)
        nc.vector.tensor_mul(s2[:], s_b[:], s_b[:])
        two_s2 = singles.tile([128, 1], F32)
        nc.vector.tensor_scalar_mul(two_s2[:], s2[:], 2.0)
        neg_two_s2 = singles.tile([128, 1], F32)
        nc.vector.tensor_scalar_mul(neg_two_s2[:], s2[:], -2.0)

        # ---- prototypes: normalize rows (-> bf16), xbar-transpose ----
        # pnTs[p, sg, jc, k, c] = pn_hat[512*sg + 128*jc + c, 128*k + p]
        pnTs = singles.tile([128, 2, 4, KC, 128], BF16)
        for sg in range(2):
            pt = pstage.tile([128, 4, D], F32, tag="pt")
            nc.sync.dma_start(
                out=pt[:],
                in_=p_dram[ts(sg, 512), :].rearrange("(j p) d -> p j d", p=128),
            )
            pn2 = small.tile([128, 4], F32, tag="pn2")
            for jc in range(4):
                sq = psq.tile([128, D], F32, tag="psq")
                nc.scalar.activation(
                    sq[:], pt[:, jc, :], mybir.ActivationFunctionType.Square,
                    accum_out=pn2[:, jc : jc + 1],
                )
            nc.scalar.sqrt(pn2[:], pn2[:])
            nc.vector.tensor_scalar_max(pn2[:], pn2[:], 1e-12)
            prinv = small.tile([128, 4], F32, tag="prinv")
            nc.vector.reciprocal(prinv[:], pn2[:])
            pnb = pnbp.tile([128, 4, D], BF16, tag="pnb")
            for jc in range(4):
                nc.vector.tensor_scalar_mul(
                    pnb[:, jc, :], pt[:, jc, :], prinv[:, jc : jc + 1]
                )
            nc.sync.dma_start(out=pnTs[:, sg, :, :, :], in_=pnb[:], transpose=True)
        # re-layout to contiguous [128, k, c:1024] so matmul rhs slices are
        # dense 2D (strided 3D moving operands stream slower on the PE)
        pnT = singles.tile([128, KC, 2 * 512], BF16)
        for k in range(KC):
            nc.vector.tensor_copy(
                out=pnT[:, k, :].rearrange("p (sg jc c) -> p sg jc c", sg=2, jc=4),
                in_=pnTs[:, :, :, k, :],
            )

        # ---- main loop over 16 groups of 4x128 feature rows ----
        for g in range(NG):
            ft = fpool.tile([128, G, D], F32, tag="ft")
            nc.gpsimd.dma_start(
                out=ft[:],
                in_=f_dram[ts(g, G * 128), :].rearrange("(j p) d -> p j d", p=128),
            )

            fb = fbpool.tile([128, G, D], BF16, tag="fb")
            n2g = small.tile([128, G], F32, tag="n2g")
            nc.vector.tensor_copy(out=fb[:], in_=ft[:])
            for j in range(G):
                sq = sqpool.tile([128, D], BF16, tag="sq")
                nc.vector.scalar_tensor_tensor(
                    out=sq[:], in0=fb[:, j, :], scalar=1.0, in1=fb[:, j, :],
                    op0=mult, op1=mult, accum_out=n2g[:, j : j + 1],
                )
            nc.scalar.sqrt(n2g[:], n2g[:])
            nc.vector.tensor_scalar_max(n2g[:], n2g[:], 1e-12)
            rinvg = small.tile([128, G], F32, tag="rinvg")
            nc.vector.reciprocal(rinvg[:], n2g[:])
            scaleg = small.tile([128, G], F32, tag="scaleg")
            nc.vector.tensor_scalar_mul(scaleg[:], rinvg[:], neg_two_s2[:])

            # one xbar transpose for the whole group: 16 contiguous [128,128]
            # chunk-transposes; chunk 4j+k = (block j, d-chunk k)
            fT = ftpool.tile([128, KC * G, 128], BF16, tag="fT")
            nc.sync.dma_start(out=fT[:], in_=fb[:], transpose=True)

            ot = opool.tile([128, G, C], F32, tag="ot")
            for j in range(G):
                dots = mpsum.tile([128, CP], F32)
                for k in range(KC):
                    for h in range(2):
                        nc.tensor.matmul(
                            dots[:, ts(h, 512)],
                            fT[:, KC * j + k, :],
                            pnT[:, k, ts(h, 512)],
                            start=(k == 0),
                            stop=(k == KC - 1),
                            skip_group_check=True,
                        )
                nc.scalar.activation(
                    ot[:, j, :], dots[:, 0:C], mybir.ActivationFunctionType.Sqrt,
                    bias=two_s2[:], scale=scaleg[:, j : j + 1],
                )
            # negate on DVE, whole group in one op: gpsimd tensor ops are
            # Q7-software (~10 G elem/s) and saturate SBUF, starving the DVE
            nc.vector.tensor_scalar_mul(ot[:], ot[:], -1.0)

            nc.scalar.dma_start(
                out=o_dram[ts(g, G * 128), :].rearrange("(j p) c -> p j c", p=128),
                in_=ot[:],
            )


def build():
    nc = bacc.Bacc("TRN2", target_bir_lowering=False, debug=False,
                   num_devices=N_CORES)
    _emit(nc)
    nc.compile()
    return nc


def _ensure_ntff_hook():
    """Dev-only: restore the axon NTFF profile hook that the trimmed agent
    image's antenv package lacks, so trace=True yields real HW timings."""
    import sys
    import types

    try:
        from antenv.axon_hooks import get_axon_ntff_profile_hook  # noqa: F401
        return
    except ImportError:
        pass
    from trn_agent_boot.trn_boot import _ntff_profile_via_ctypes

    hook = _ntff_profile_via_ctypes("/opt/axon/libaxon_pjrt.so")
    mod = types.ModuleType("antenv.axon_hooks")
    mod.get_axon_ntff_profile_hook = lambda: hook
    mod.set_axon_ntff_profile_hook = lambda h: None
    sys.modules["antenv.axon_hooks"] = mod


def run(inputs, trace=False):
    if trace:
        _ensure_ntff_hook()
    feats = np.ascontiguousarray(np.asarray(inputs["features"], dtype=np.float32))
    protos = np.ascontiguousarray(np.asarray(inputs["prototypes"], dtype=np.float32))
    dscale = np.ascontiguousarray(np.asarray(inputs["distance_scale"], dtype=np.float32))
    protos_p = np.zeros((CP, D), dtype=np.float32)
    protos_p[:C] = protos
    nc = build()
    in_maps = [
        {
            "features": feats[i * BS : (i + 1) * BS],
            "prototypes": protos_p,
            "distance_scale": dscale,
        }
        for i in range(N_CORES)
    ]
    res = run_bass_kernel_spmd(nc, in_maps, core_ids=list(range(N_CORES)),
                               trace=trace)
    out = np.concatenate([r["out"] for r in res.results], axis=0)
    return out, res


def kernel(**inputs) -> np.ndarray:
    out, _ = run(inputs, trace=False)
    return out
